# revision 5
# baseline (speedup 1.0000x reference)
"""Trainium2 Bass kernel for nn_Net_20091857011309.

Two independent 4096-step GRU chains (D=1024, H=2048) + small MLP head.

Strategy: block-Jacobi fixed-point iteration over blocks of B=512 timesteps.
Each iteration evaluates all timesteps of the block in parallel as a GEMM
(h-projections for the whole block), then applies the GRU gate math
elementwise, using the previous iterate's hidden states shifted by one step.
Information propagates one timestep per iteration and the GRU Jacobian
contracts at ~0.62x/iter for these weights, so K iterations give ~0.62^K
worst-case error (K=24 -> ~3e-5 absolute on h, |h|<=0.78).

Sharding: both chains run on all 8 cores. The 3H=6144 gate dimension is
sharded 8 ways (each core owns rows [256j,256j+256) of each of the r/z/n
blocks). Per iteration each core computes its [768, 512] gate slab
(fp16 matmuls, fp32 accumulate), the gate math, and its [256, 512] h_new
slice; one AllGather per chain per iteration rebuilds the full [2048, 512]
H block on every core. The two chains' iterations are interleaved so each
chain's collective+DMA tail hides under the other chain's matmuls.
"""

import numpy as np

H = 2048
D = 1024
T = 4096
N_CORES = 8
B = 512            # Jacobi block length (timesteps)
K_ITERS = 24       # Jacobi iterations per block
NBLK = T // B
SH = H // N_CORES  # 256 h-rows owned per core
SG = 3 * SH        # 768 gate rows per core (r,z,n slices)
MT = SG // 128     # 6 m-tiles (0,1=r; 2,3=z; 4,5=n)
KT = H // 128      # 16 k-chunks over the h (contraction) dim
DT = D // 128      # 8 k-chunks over the input dim
FCK = 2 * H // 128  # 32 k-chunks for fc1

_CACHE = {}


def _build_module():
    import concourse.mybir as mybir
    import concourse.tile as tile
    from concourse import bacc

    dt = mybir.dt
    F16, F32 = dt.float16, dt.float32
    AF = mybir.ActivationFunctionType
    ALU = mybir.AluOpType

    nc = bacc.Bacc("TRN2", target_bir_lowering=False, debug=False,
                   num_devices=N_CORES)

    chains = ("A", "B")
    whh_t = {c: nc.dram_tensor(f"whhT_{c}", [H, SG], F16, kind="ExternalInput") for c in chains}
    wih_t = {c: nc.dram_tensor(f"wihT_{c}", [D, SG], F16, kind="ExternalInput") for c in chains}
    xT_t = {c: nc.dram_tensor(f"xT_{c}", [D, T], F16, kind="ExternalInput") for c in chains}
    bxp_t = {c: nc.dram_tensor(f"bxp_{c}", [SG], F32, kind="ExternalInput") for c in chains}
    bhn_t = {c: nc.dram_tensor(f"bhn_{c}", [SH], F32, kind="ExternalInput") for c in chains}
    fc1w_t = nc.dram_tensor("fc1wT", [2 * H, 256], F16, kind="ExternalInput")
    fc1b_t = nc.dram_tensor("fc1b", [256], F32, kind="ExternalInput")
    fc2w_t = nc.dram_tensor("fc2wT", [256, 3], F32, kind="ExternalInput")
    fc2b_t = nc.dram_tensor("fc2b", [1, 3], F32, kind="ExternalInput")
    out_t = nc.dram_tensor("out", [1, 3], F32, kind="ExternalOutput")

    with tile.TileContext(nc) as tc:
        with (
            tc.tile_pool(name="persist", bufs=1) as persist,
            tc.tile_pool(name="dram", bufs=1, space="DRAM") as dram,
        ):
            # ---- persistent SBUF state ----
            whh_sb, wih_sb, H_sb, xp_sb, hprev_sb, hnew_sb = {}, {}, {}, {}, {}, {}
            bxp_sb, bhn_sb, ag_in, ag_out = {}, {}, {}, {}
            for c in chains:
                whh_sb[c] = persist.tile([128, KT, SG], F16, name=f"whh_sb_{c}")
                wih_sb[c] = persist.tile([128, DT, SG], F16, name=f"wih_sb_{c}")
                H_sb[c] = persist.tile([128, KT, B + 1], F16, name=f"H_sb_{c}")
                xp_sb[c] = persist.tile([128, MT, B], F32, name=f"xp_sb_{c}")
                hprev_sb[c] = persist.tile([128, 2, B], F32, name=f"hprev_sb_{c}")
                hnew_sb[c] = persist.tile([128, 2, B], F16, name=f"hnew_sb_{c}")
                bxp_sb[c] = persist.tile([128, MT], F32, name=f"bxp_sb_{c}")
                bhn_sb[c] = persist.tile([128, 2], F32, name=f"bhn_sb_{c}")
                ag_in[c] = dram.tile([SH, B], F16, name=f"ag_in_{c}")

                nc.sync.dma_start(whh_sb[c][:], whh_t[c].rearrange("(k p) m -> p k m", p=128))
                nc.sync.dma_start(wih_sb[c][:], wih_t[c].rearrange("(k p) m -> p k m", p=128))
                nc.sync.dma_start(bxp_sb[c][:], bxp_t[c].rearrange("(m p) -> p m", p=128))
                nc.sync.dma_start(bhn_sb[c][:], bhn_t[c].rearrange("(m p) -> p m", p=128))
                nc.vector.memset(H_sb[c][:], 0.0)
                nc.vector.memset(hprev_sb[c][:], 0.0)
                nc.vector.memset(hnew_sb[c][:], 0.0)

            with (
                tc.tile_pool(name="work", bufs=2) as work,
                tc.tile_pool(name="xstage", bufs=2) as xstage,
                tc.tile_pool(name="psum", bufs=4, space="PSUM") as psum,
            ):
                for blk in range(NBLK):
                    b0 = blk * B
                    # ---- per-block setup + input projections ----
                    for c in chains:
                        if blk > 0:
                            # carry boundary column: h_start = last h of prev block
                            nc.vector.tensor_copy(H_sb[c][:, :, 0:1], H_sb[c][:, :, B:B + 1])
                            nc.vector.tensor_copy(hprev_sb[c][:, :, 0:1], hnew_sb[c][:, :, B - 1:B])
                            # init H^(0) = broadcast(h_start) along the block
                            for k in range(KT):
                                nc.scalar.activation(
                                    H_sb[c][:, k, 1:B + 1], H_sb[c][:, k, 1:B + 1],
                                    AF.Identity, bias=H_sb[c][:, k, 0:1], scale=0.0)
                            for mi in range(2):
                                nc.scalar.activation(
                                    hprev_sb[c][:, mi, 1:B], hprev_sb[c][:, mi, 1:B],
                                    AF.Identity, bias=hprev_sb[c][:, mi, 0:1], scale=0.0)

                        # xp = W_ih @ x.T + bias for this block: [SG, B]
                        xb = xstage.tile([128, DT, B], F16, name="xb")
                        nc.sync.dma_start(xb[:], xT_t[c].rearrange("(k p) n -> p k n", p=128)[:, :, b0:b0 + B])
                        for m in range(MT):
                            ps = psum.tile([128, B], F32, name="ps")
                            for k in range(DT):
                                nc.tensor.matmul(
                                    ps[:], wih_sb[c][:, k, 128 * m:128 * (m + 1)], xb[:, k, :],
                                    start=(k == 0), stop=(k == DT - 1))
                            nc.scalar.activation(xp_sb[c][:, m, :], ps[:], AF.Identity,
                                                 bias=bxp_sb[c][:, m:m + 1])

                    # ---- Jacobi iterations (chains interleaved) ----
                    for it in range(K_ITERS):
                        for c in chains:
                            g = []  # gate PSUM slabs m=0..5
                            for m in range(MT):
                                ps = psum.tile([128, B], F32, name="ps")
                                for k in range(KT):
                                    nc.tensor.matmul(
                                        ps[:], whh_sb[c][:, k, 128 * m:128 * (m + 1)],
                                        H_sb[c][:, k, 0:B],
                                        start=(k == 0), stop=(k == KT - 1))
                                g.append(ps)
                            # consume PSUM slabs in production order: g0,g1 (r), g2,g3 (z), g4,g5 (n)
                            r, z, n = [], [], []
                            for mi in range(2):
                                pre = work.tile([128, B], F32, name="tt", bufs=4)
                                nc.vector.tensor_add(pre[:], g[mi][:], xp_sb[c][:, mi, :])
                                r.append(work.tile([128, B], F32, name="r", bufs=3))
                                nc.scalar.activation(r[mi][:], pre[:], AF.Sigmoid)
                            for mi in range(2):
                                pre = work.tile([128, B], F32, name="tt", bufs=4)
                                nc.vector.tensor_add(pre[:], g[2 + mi][:], xp_sb[c][:, 2 + mi, :])
                                z.append(work.tile([128, B], F32, name="z", bufs=3))
                                nc.scalar.activation(z[mi][:], pre[:], AF.Sigmoid)
                            for mi in range(2):
                                # tmp = r * (g_n + b_hh_n)
                                tmp = work.tile([128, B], F32, name="tt", bufs=4)
                                nc.vector.scalar_tensor_tensor(
                                    tmp[:], g[4 + mi][:], bhn_sb[c][:, mi:mi + 1], r[mi][:],
                                    op0=ALU.add, op1=ALU.mult)
                                pre = work.tile([128, B], F32, name="tt", bufs=4)
                                nc.vector.tensor_add(pre[:], tmp[:], xp_sb[c][:, 4 + mi, :])
                                n.append(work.tile([128, B], F32, name="n", bufs=3))
                                nc.scalar.activation(n[mi][:], pre[:], AF.Tanh)
                            for mi in range(2):
                                # h_new = n + z * (hprev - n)
                                t1 = work.tile([128, B], F32, name="tt", bufs=4)
                                nc.vector.tensor_sub(t1[:], hprev_sb[c][:, mi, :], n[mi][:])
                                t2 = work.tile([128, B], F32, name="tt", bufs=4)
                                nc.vector.tensor_mul(t2[:], t1[:], z[mi][:])
                                nc.vector.tensor_add(hnew_sb[c][:, mi, :], t2[:], n[mi][:])
                                # own shifted copy for next iteration's hprev
                                nc.vector.tensor_copy(hprev_sb[c][:, mi, 1:B], hnew_sb[c][:, mi, 0:B - 1])

                            # exchange h_new slices -> full H block
                            # (Shared tensors are single-writer: fresh tile per AG, slots rotate)
                            ag_o = dram.tile([H, B], F16, addr_space="Shared", name="ag_o", bufs=2)
                            nc.sync.dma_start(ag_in[c].rearrange("(q p) n -> p q n", p=128), hnew_sb[c][:])
                            nc.gpsimd.collective_compute(
                                "AllGather", ALU.bypass,
                                replica_groups=[list(range(N_CORES))],
                                ins=[ag_in[c][:].opt()],
                                outs=[ag_o[:].opt()])
                            nc.sync.dma_start(H_sb[c][:, :, 1:B + 1], ag_o.rearrange("(k p) n -> p k n", p=128))

            # ---- MLP head (identical on every core) ----
            with (
                tc.tile_pool(name="mlp", bufs=1) as mlp,
                tc.tile_pool(name="mlp_ps", bufs=2, space="PSUM") as mlp_ps,
            ):
                fc1w_sb = mlp.tile([128, FCK, 256], F16, name="fc1w_sb")
                nc.sync.dma_start(fc1w_sb[:], fc1w_t.rearrange("(k p) m -> p k m", p=128))
                fc1b_sb = mlp.tile([128, 2], F32, name="fc1b_sb")
                nc.sync.dma_start(fc1b_sb[:], fc1b_t.rearrange("(m p) -> p m", p=128))
                fc2w_sb = mlp.tile([128, 2, 3], F32, name="fc2w_sb")
                nc.sync.dma_start(fc2w_sb[:], fc2w_t.rearrange("(m p) n -> p m n", p=128))
                fc2b_sb = mlp.tile([1, 3], F32, name="fc2b_sb")
                nc.sync.dma_start(fc2b_sb[:], fc2b_t[:, :])

                o1_sb = mlp.tile([128, 2], F32, name="o1_sb")
                for mi in range(2):
                    ps1 = mlp_ps.tile([128, 1], F32, name="ps1")
                    for kk in range(FCK):
                        src = H_sb["A"] if kk < KT else H_sb["B"]
                        nc.tensor.matmul(
                            ps1[:], fc1w_sb[:, kk, 128 * mi:128 * (mi + 1)],
                            src[:, kk % KT, B:B + 1],
                            start=(kk == 0), stop=(kk == FCK - 1))
                    nc.scalar.activation(o1_sb[:, mi:mi + 1], ps1[:], AF.Relu,
                                         bias=fc1b_sb[:, mi:mi + 1])

                ps2 = mlp_ps.tile([1, 3], F32, name="ps2")
                for mi in range(2):
                    nc.tensor.matmul(ps2[:], o1_sb[:, mi:mi + 1], fc2w_sb[:, mi, :],
                                     start=(mi == 0), stop=(mi == 1))
                logits = mlp.tile([1, 3], F32, name="logits")
                nc.vector.tensor_add(logits[:], ps2[:], fc2b_sb[:])

                # log_softmax along the free dim
                mx = mlp.tile([1, 1], F32, name="mx")
                nc.vector.tensor_reduce(mx[:], logits[:], mybir.AxisListType.X, ALU.max)
                tshift = mlp.tile([1, 3], F32, name="tshift")
                nc.vector.tensor_scalar_sub(tshift[:], logits[:], mx[:])
                ex = mlp.tile([1, 3], F32, name="ex")
                nc.scalar.activation(ex[:], tshift[:], AF.Exp)
                ssum = mlp.tile([1, 1], F32, name="ssum")
                nc.vector.tensor_reduce(ssum[:], ex[:], mybir.AxisListType.X, ALU.add)
                lse = mlp.tile([1, 1], F32, name="lse")
                nc.scalar.activation(lse[:], ssum[:], AF.Ln)
                res = mlp.tile([1, 3], F32, name="res")
                nc.vector.tensor_scalar_sub(res[:], tshift[:], lse[:])
                nc.sync.dma_start(out_t[:, :], res[:])

    nc.compile()
    return nc


def _prep_inputs(inputs):
    """Build the 8 per-core input maps from the full problem inputs."""
    f16, f32 = np.float16, np.float32
    x = {"A": np.asarray(inputs["x1"]), "B": np.asarray(inputs["x2"])}
    W_ih = {"A": np.asarray(inputs["W_ih1"]), "B": np.asarray(inputs["W_ih2"])}
    W_hh = {"A": np.asarray(inputs["W_hh1"]), "B": np.asarray(inputs["W_hh2"])}
    b_ih = {"A": np.asarray(inputs["b_ih1"]), "B": np.asarray(inputs["b_ih2"])}
    b_hh = {"A": np.asarray(inputs["b_hh1"]), "B": np.asarray(inputs["b_hh2"])}

    shared = {
        "fc1wT": np.ascontiguousarray(np.asarray(inputs["fc1_w"]).T).astype(f16),
        "fc1b": np.asarray(inputs["fc1_b"]).astype(f32),
        "fc2wT": np.ascontiguousarray(np.asarray(inputs["fc2_w"]).T).astype(f32),
        "fc2b": np.asarray(inputs["fc2_b"]).astype(f32).reshape(1, 3),
    }
    xTs = {c: np.ascontiguousarray(x[c].T).astype(f16) for c in "AB"}

    in_maps = []
    for j in range(N_CORES):
        m = dict(shared)
        sl = slice(SH * j, SH * (j + 1))
        for c in "AB":
            rows = np.r_[np.arange(SH * j, SH * (j + 1)),
                         np.arange(H + SH * j, H + SH * (j + 1)),
                         np.arange(2 * H + SH * j, 2 * H + SH * (j + 1))]
            m[f"whhT_{c}"] = np.ascontiguousarray(W_hh[c][rows].T).astype(f16)
            m[f"wihT_{c}"] = np.ascontiguousarray(W_ih[c][rows].T).astype(f16)
            bxp = b_ih[c][rows].astype(f32).copy()
            bxp[:SH] += b_hh[c][:H][sl]
            bxp[SH:2 * SH] += b_hh[c][H:2 * H][sl]
            m[f"bxp_{c}"] = bxp
            m[f"bhn_{c}"] = b_hh[c][2 * H:][sl].astype(f32)
            m[f"xT_{c}"] = xTs[c]
        in_maps.append(m)
    return in_maps


def kernel(**inputs) -> np.ndarray:
    from concourse.bass_utils import run_bass_kernel_spmd

    if "nc" not in _CACHE:
        _CACHE["nc"] = _build_module()
    nc = _CACHE["nc"]
    in_maps = _prep_inputs(inputs)
    res = run_bass_kernel_spmd(nc, in_maps, core_ids=list(range(N_CORES)))
    return np.asarray(res.results[0]["out"], dtype=np.float32)


# revision 7
# speedup vs baseline: 1.0126x; 1.0126x over previous
"""Trainium2 Bass kernel for nn_Net_20091857011309.

Two independent 4096-step GRU chains (D=1024, H=2048) + small MLP head.

Strategy: block-Jacobi fixed-point iteration over blocks of B=512 timesteps.
Each iteration evaluates all timesteps of the block in parallel as a GEMM
(h-projections for the whole block), then applies the GRU gate math
elementwise, using the previous iterate's hidden states shifted by one step.
Information propagates one timestep per iteration and the GRU Jacobian
contracts at ~0.62x/iter for these weights, so K iterations give ~0.62^K
worst-case error (K=24 -> ~3e-5 absolute on h, |h|<=0.78).

Sharding: both chains run on all 8 cores. The 3H=6144 gate dimension is
sharded 8 ways (each core owns rows [256j,256j+256) of each of the r/z/n
blocks). Per iteration each core computes its [768, 512] gate slab
(fp16 matmuls, fp32 accumulate), the gate math, and its [256, 512] h_new
slice; one AllGather per chain per iteration rebuilds the full [2048, 512]
H block on every core. The two chains' iterations are interleaved so each
chain's collective+DMA tail hides under the other chain's matmuls.
"""

import os
import numpy as np

H = 2048
D = 1024
T = 4096
N_CORES = 8
B = 512            # Jacobi block length (timesteps)
K_ITERS = int(os.environ.get("GRU_K_ITERS", "24"))   # Jacobi iterations per block
_WITH_AG = os.environ.get("GRU_WITH_AG", "1") == "1"  # debug knob (timing only)
NBLK = T // B
SH = H // N_CORES  # 256 h-rows owned per core
SG = 3 * SH        # 768 gate rows per core (r,z,n slices)
MT = SG // 128     # 6 m-tiles (0,1=r; 2,3=z; 4,5=n)
KT = H // 128      # 16 k-chunks over the h (contraction) dim
DT = D // 128      # 8 k-chunks over the input dim
FCK = 2 * H // 128  # 32 k-chunks for fc1

_CACHE = {}


def _build_module():
    import concourse.mybir as mybir
    import concourse.tile as tile
    from concourse import bacc

    dt = mybir.dt
    F16, F32 = dt.float16, dt.float32
    AF = mybir.ActivationFunctionType
    ALU = mybir.AluOpType

    nc = bacc.Bacc("TRN2", target_bir_lowering=False, debug=False,
                   num_devices=N_CORES)

    chains = ("A", "B")
    whh_t = {c: nc.dram_tensor(f"whhT_{c}", [H, SG], F16, kind="ExternalInput") for c in chains}
    wih_t = {c: nc.dram_tensor(f"wihT_{c}", [D, SG], F16, kind="ExternalInput") for c in chains}
    xT_t = {c: nc.dram_tensor(f"xT_{c}", [D, T], F16, kind="ExternalInput") for c in chains}
    bxp_t = {c: nc.dram_tensor(f"bxp_{c}", [SG], F32, kind="ExternalInput") for c in chains}
    bhn_t = {c: nc.dram_tensor(f"bhn_{c}", [SH], F32, kind="ExternalInput") for c in chains}
    fc1w_t = nc.dram_tensor("fc1wT", [2 * H, 256], F16, kind="ExternalInput")
    fc1b_t = nc.dram_tensor("fc1b", [256], F32, kind="ExternalInput")
    fc2w_t = nc.dram_tensor("fc2wT", [256, 3], F32, kind="ExternalInput")
    fc2b_t = nc.dram_tensor("fc2b", [1, 3], F32, kind="ExternalInput")
    out_t = nc.dram_tensor("out", [1, 3], F32, kind="ExternalOutput")

    with tile.TileContext(nc) as tc:
        with (
            tc.tile_pool(name="persist", bufs=1) as persist,
            tc.tile_pool(name="dram", bufs=1, space="DRAM") as dram,
        ):
            # ---- persistent SBUF state ----
            whh_sb, wih_sb, H_sb, xp_sb, hprev_sb, hnew_sb = {}, {}, {}, {}, {}, {}
            bxp_sb, bhn_sb, ag_in, ag_out = {}, {}, {}, {}
            for c in chains:
                whh_sb[c] = persist.tile([128, KT, SG], F16, name=f"whh_sb_{c}")
                wih_sb[c] = persist.tile([128, DT, SG], F16, name=f"wih_sb_{c}")
                H_sb[c] = persist.tile([128, KT, B + 1], F16, name=f"H_sb_{c}")
                xp_sb[c] = persist.tile([128, MT, B], F32, name=f"xp_sb_{c}")
                hprev_sb[c] = persist.tile([128, 2, B], F32, name=f"hprev_sb_{c}")
                hnew_sb[c] = persist.tile([128, 2, B], F16, name=f"hnew_sb_{c}")
                bxp_sb[c] = persist.tile([128, MT], F32, name=f"bxp_sb_{c}")
                bhn_sb[c] = persist.tile([128, 2], F32, name=f"bhn_sb_{c}")
                ag_in[c] = dram.tile([SH, B], F16, name=f"ag_in_{c}")

                nc.sync.dma_start(whh_sb[c][:], whh_t[c].rearrange("(k p) m -> p k m", p=128))
                nc.sync.dma_start(wih_sb[c][:], wih_t[c].rearrange("(k p) m -> p k m", p=128))
                nc.sync.dma_start(bxp_sb[c][:], bxp_t[c].rearrange("(m p) -> p m", p=128))
                nc.sync.dma_start(bhn_sb[c][:], bhn_t[c].rearrange("(m p) -> p m", p=128))
                nc.vector.memset(H_sb[c][:], 0.0)
                nc.vector.memset(hprev_sb[c][:], 0.0)
                nc.vector.memset(hnew_sb[c][:], 0.0)

            with (
                tc.tile_pool(name="work", bufs=2) as work,
                tc.tile_pool(name="xstage", bufs=2) as xstage,
                tc.tile_pool(name="psum", bufs=4, space="PSUM") as psum,
            ):
                for blk in range(NBLK):
                    b0 = blk * B
                    # ---- per-block setup + input projections ----
                    for c in chains:
                        if blk > 0:
                            # carry boundary column: h_start = last h of prev block
                            nc.vector.tensor_copy(H_sb[c][:, :, 0:1], H_sb[c][:, :, B:B + 1])
                            nc.vector.tensor_copy(hprev_sb[c][:, :, 0:1], hnew_sb[c][:, :, B - 1:B])
                            # init H^(0) = broadcast(h_start) along the block
                            for k in range(KT):
                                nc.scalar.activation(
                                    H_sb[c][:, k, 1:B + 1], H_sb[c][:, k, 1:B + 1],
                                    AF.Identity, bias=H_sb[c][:, k, 0:1], scale=0.0)
                            for mi in range(2):
                                nc.scalar.activation(
                                    hprev_sb[c][:, mi, 1:B], hprev_sb[c][:, mi, 1:B],
                                    AF.Identity, bias=hprev_sb[c][:, mi, 0:1], scale=0.0)

                        # xp = W_ih @ x.T + bias for this block: [SG, B]
                        xb = xstage.tile([128, DT, B], F16, name="xb")
                        nc.sync.dma_start(xb[:], xT_t[c].rearrange("(k p) n -> p k n", p=128)[:, :, b0:b0 + B])
                        for m in range(MT):
                            ps = psum.tile([128, B], F32, name="ps")
                            for k in range(DT):
                                nc.tensor.matmul(
                                    ps[:], wih_sb[c][:, k, 128 * m:128 * (m + 1)], xb[:, k, :],
                                    start=(k == 0), stop=(k == DT - 1))
                            nc.scalar.activation(xp_sb[c][:, m, :], ps[:], AF.Identity,
                                                 bias=bxp_sb[c][:, m:m + 1])

                    # ---- Jacobi iterations (chains interleaved) ----
                    for it in range(K_ITERS):
                        for c in chains:
                            g = []  # gate PSUM slabs m=0..5
                            for m in range(MT):
                                ps = psum.tile([128, B], F32, name="ps")
                                for k in range(KT):
                                    nc.tensor.matmul(
                                        ps[:], whh_sb[c][:, k, 128 * m:128 * (m + 1)],
                                        H_sb[c][:, k, 0:B],
                                        start=(k == 0), stop=(k == KT - 1))
                                g.append(ps)
                            # consume PSUM slabs in production order: g0,g1 (r), g2,g3 (z), g4,g5 (n)
                            r, z, n = [], [], []
                            for mi in range(2):
                                pre = work.tile([128, B], F32, name="tt", bufs=4)
                                nc.vector.tensor_add(pre[:], g[mi][:], xp_sb[c][:, mi, :])
                                r.append(work.tile([128, B], F32, name="r", bufs=3))
                                nc.scalar.activation(r[mi][:], pre[:], AF.Sigmoid)
                            for mi in range(2):
                                pre = work.tile([128, B], F32, name="tt", bufs=4)
                                nc.vector.tensor_add(pre[:], g[2 + mi][:], xp_sb[c][:, 2 + mi, :])
                                z.append(work.tile([128, B], F32, name="z", bufs=3))
                                nc.scalar.activation(z[mi][:], pre[:], AF.Sigmoid)
                            for mi in range(2):
                                # tmp = r * (g_n + b_hh_n)
                                tmp = work.tile([128, B], F32, name="tt", bufs=4)
                                nc.vector.scalar_tensor_tensor(
                                    tmp[:], g[4 + mi][:], bhn_sb[c][:, mi:mi + 1], r[mi][:],
                                    op0=ALU.add, op1=ALU.mult)
                                pre = work.tile([128, B], F32, name="tt", bufs=4)
                                nc.vector.tensor_add(pre[:], tmp[:], xp_sb[c][:, 4 + mi, :])
                                n.append(work.tile([128, B], F32, name="n", bufs=3))
                                nc.scalar.activation(n[mi][:], pre[:], AF.Tanh)
                            for mi in range(2):
                                # h_new = n + z * (hprev - n)
                                t1 = work.tile([128, B], F32, name="tt", bufs=4)
                                nc.vector.tensor_sub(t1[:], hprev_sb[c][:, mi, :], n[mi][:])
                                t2 = work.tile([128, B], F32, name="tt", bufs=4)
                                nc.vector.tensor_mul(t2[:], t1[:], z[mi][:])
                                nc.vector.tensor_add(hnew_sb[c][:, mi, :], t2[:], n[mi][:])
                                # own shifted copy for next iteration's hprev
                                nc.vector.tensor_copy(hprev_sb[c][:, mi, 1:B], hnew_sb[c][:, mi, 0:B - 1])

                            # exchange h_new slices -> full H block
                            # (Shared tensors are single-writer: fresh tile per AG, slots rotate)
                            nc.sync.dma_start(ag_in[c].rearrange("(q p) n -> p q n", p=128), hnew_sb[c][:])
                            if _WITH_AG:
                                ag_o = dram.tile([H, B], F16, addr_space="Shared", name="ag_o", bufs=2)
                                nc.gpsimd.collective_compute(
                                    "AllGather", ALU.bypass,
                                    replica_groups=[list(range(N_CORES))],
                                    ins=[ag_in[c][:].opt()],
                                    outs=[ag_o[:].opt()])
                                nc.sync.dma_start(H_sb[c][:, :, 1:B + 1], ag_o.rearrange("(k p) n -> p k n", p=128))

            # ---- MLP head (identical on every core) ----
            with (
                tc.tile_pool(name="mlp", bufs=1) as mlp,
                tc.tile_pool(name="mlp_ps", bufs=2, space="PSUM") as mlp_ps,
            ):
                fc1w_sb = mlp.tile([128, FCK, 256], F16, name="fc1w_sb")
                nc.sync.dma_start(fc1w_sb[:], fc1w_t.rearrange("(k p) m -> p k m", p=128))
                fc1b_sb = mlp.tile([128, 2], F32, name="fc1b_sb")
                nc.sync.dma_start(fc1b_sb[:], fc1b_t.rearrange("(m p) -> p m", p=128))
                fc2w_sb = mlp.tile([128, 2, 3], F32, name="fc2w_sb")
                nc.sync.dma_start(fc2w_sb[:], fc2w_t.rearrange("(m p) n -> p m n", p=128))
                fc2b_sb = mlp.tile([1, 3], F32, name="fc2b_sb")
                nc.sync.dma_start(fc2b_sb[:], fc2b_t[:, :])

                o1_sb = mlp.tile([128, 2], F32, name="o1_sb")
                for mi in range(2):
                    ps1 = mlp_ps.tile([128, 1], F32, name="ps1")
                    for kk in range(FCK):
                        src = H_sb["A"] if kk < KT else H_sb["B"]
                        nc.tensor.matmul(
                            ps1[:], fc1w_sb[:, kk, 128 * mi:128 * (mi + 1)],
                            src[:, kk % KT, B:B + 1],
                            start=(kk == 0), stop=(kk == FCK - 1))
                    nc.scalar.activation(o1_sb[:, mi:mi + 1], ps1[:], AF.Relu,
                                         bias=fc1b_sb[:, mi:mi + 1])

                ps2 = mlp_ps.tile([1, 3], F32, name="ps2")
                for mi in range(2):
                    nc.tensor.matmul(ps2[:], o1_sb[:, mi:mi + 1], fc2w_sb[:, mi, :],
                                     start=(mi == 0), stop=(mi == 1))
                logits = mlp.tile([1, 3], F32, name="logits")
                nc.vector.tensor_add(logits[:], ps2[:], fc2b_sb[:])

                # log_softmax along the free dim
                mx = mlp.tile([1, 1], F32, name="mx")
                nc.vector.tensor_reduce(mx[:], logits[:], mybir.AxisListType.X, ALU.max)
                tshift = mlp.tile([1, 3], F32, name="tshift")
                nc.vector.tensor_scalar_sub(tshift[:], logits[:], mx[:])
                ex = mlp.tile([1, 3], F32, name="ex")
                nc.scalar.activation(ex[:], tshift[:], AF.Exp)
                ssum = mlp.tile([1, 1], F32, name="ssum")
                nc.vector.tensor_reduce(ssum[:], ex[:], mybir.AxisListType.X, ALU.add)
                lse = mlp.tile([1, 1], F32, name="lse")
                nc.scalar.activation(lse[:], ssum[:], AF.Ln)
                res = mlp.tile([1, 3], F32, name="res")
                nc.vector.tensor_scalar_sub(res[:], tshift[:], lse[:])
                nc.sync.dma_start(out_t[:, :], res[:])

    nc.compile()
    return nc


def _prep_inputs(inputs):
    """Build the 8 per-core input maps from the full problem inputs."""
    f16, f32 = np.float16, np.float32
    x = {"A": np.asarray(inputs["x1"]), "B": np.asarray(inputs["x2"])}
    W_ih = {"A": np.asarray(inputs["W_ih1"]), "B": np.asarray(inputs["W_ih2"])}
    W_hh = {"A": np.asarray(inputs["W_hh1"]), "B": np.asarray(inputs["W_hh2"])}
    b_ih = {"A": np.asarray(inputs["b_ih1"]), "B": np.asarray(inputs["b_ih2"])}
    b_hh = {"A": np.asarray(inputs["b_hh1"]), "B": np.asarray(inputs["b_hh2"])}

    shared = {
        "fc1wT": np.ascontiguousarray(np.asarray(inputs["fc1_w"]).T).astype(f16),
        "fc1b": np.asarray(inputs["fc1_b"]).astype(f32),
        "fc2wT": np.ascontiguousarray(np.asarray(inputs["fc2_w"]).T).astype(f32),
        "fc2b": np.asarray(inputs["fc2_b"]).astype(f32).reshape(1, 3),
    }
    xTs = {c: np.ascontiguousarray(x[c].T).astype(f16) for c in "AB"}

    in_maps = []
    for j in range(N_CORES):
        m = dict(shared)
        sl = slice(SH * j, SH * (j + 1))
        for c in "AB":
            rows = np.r_[np.arange(SH * j, SH * (j + 1)),
                         np.arange(H + SH * j, H + SH * (j + 1)),
                         np.arange(2 * H + SH * j, 2 * H + SH * (j + 1))]
            m[f"whhT_{c}"] = np.ascontiguousarray(W_hh[c][rows].T).astype(f16)
            m[f"wihT_{c}"] = np.ascontiguousarray(W_ih[c][rows].T).astype(f16)
            bxp = b_ih[c][rows].astype(f32).copy()
            bxp[:SH] += b_hh[c][:H][sl]
            bxp[SH:2 * SH] += b_hh[c][H:2 * H][sl]
            m[f"bxp_{c}"] = bxp
            m[f"bhn_{c}"] = b_hh[c][2 * H:][sl].astype(f32)
            m[f"xT_{c}"] = xTs[c]
        in_maps.append(m)
    return in_maps


def kernel(**inputs) -> np.ndarray:
    from concourse.bass_utils import run_bass_kernel_spmd

    if "nc" not in _CACHE:
        _CACHE["nc"] = _build_module()
    nc = _CACHE["nc"]
    in_maps = _prep_inputs(inputs)
    res = run_bass_kernel_spmd(nc, in_maps, core_ids=list(range(N_CORES)))
    return np.asarray(res.results[0]["out"], dtype=np.float32)
